# revision 8
# baseline (speedup 1.0000x reference)
"""Trainium2 Bass kernel for nn_ControGraphConvLayer (GNN message passing).

reference:
    input_       = F_ @ X                 # [N, IN]
    new_features = input_ @ W.T + b       # [N, OUT]
    new_force    = pairwise_dist(new_features) * p2p   # [N, N]
    return (new_force, new_features)

Strategy (8 NeuronCores, row-sharded):
  - algebraic: F_ @ X @ W.T == F_ @ (X @ W.T); contract X@W.T first (Y, [N, OUT]).
  - core c holds rows [c*1024, (c+1)*1024) of F_ / p2p / outputs.
  - stage 1 (every core, tiny): Y = X @ W.T via PE-transposed X tiles, fp32.
  - stage 2: emb.T[:, local] = (Y.T @ F_local.T) + b. F tiles are PE-transposed
    (fp32r, 1.5 cyc/row) and the matmuls run in fp32r (full PE rate); the bias
    is folded into the PSUM->SBUF copy.
  - AllGather of [emb.T_local; sq_local] (65 x 1024, fp32) across the 8 cores.
  - stage 3: dist^2 comes out of a single K=66 fp32 matmul with augmented
    operands A = [-2*emb.T_local; 1; sq_local+EPS], B = [emb.T_full; sq_full; 1].
    EPS=4.0 keeps the (numerically ~0) diagonal positive so sqrt never NaNs;
    the induced absolute error (~2) is far below the output scale (~1e3).
    Epilogue: ACT sqrt (PSUM->SBUF), DVE multiply by p2p, DMA out.
"""

import sys

if "/opt/trn_rl_repo" not in sys.path:
    sys.path.insert(0, "/opt/trn_rl_repo")

import numpy as np

import concourse.bass as bass
import concourse.mybir as mybir
from concourse import bacc
from concourse.bass_utils import run_bass_kernel_spmd
from concourse.masks import make_identity
from concourse.tile import TileContext

F32 = mybir.dt.float32
F32R = mybir.dt.float32r

N = 8192
IN = 128
OUT = 64
NCORES = 8
LOC = N // NCORES  # 1024 rows per core
EPS = 4.0

_cache = {}


def _build():
    nc = bacc.Bacc(target_bir_lowering=False)

    # per-core inputs
    f = nc.declare_dram_parameter("f", [LOC, N], F32R, isOutput=False)
    x = nc.declare_dram_parameter("x", [N, IN], F32, isOutput=False)
    w = nc.declare_dram_parameter("w", [OUT, IN], F32, isOutput=False)
    bvec = nc.declare_dram_parameter("bvec", [OUT, 1], F32, isOutput=False)
    p2p = nc.declare_dram_parameter("p2p", [LOC, N], F32, isOutput=False)

    # per-core outputs
    force = nc.declare_dram_parameter("force", [LOC, N], F32, isOutput=True)
    feat = nc.declare_dram_parameter("feat", [LOC, OUT], F32, isOutput=True)

    # collective bounce buffers
    ag_in = nc.dram_tensor("ag_in", [OUT + 1, LOC], F32)
    ag_out = nc.dram_tensor("ag_out", [NCORES, OUT + 1, LOC], F32, addr_space="Shared")

    with TileContext(nc) as tc, tc.tile_pool(name="const", bufs=1) as const:
        with (
            tc.tile_pool(name="st1", bufs=4) as st1,
            tc.tile_pool(name="st1ps", bufs=2, space="PSUM") as st1ps,
        ):
            # ---- constants ----
            ident = const.tile([128, 128], F32)
            make_identity(nc, ident)
            ident_r = const.tile([128, 128], F32R)
            nc.vector.tensor_copy(ident_r[:, :], ident[:, :])
            ones_col = const.tile([OUT, 1], F32)
            nc.vector.memset(ones_col[:, :], 1.0)
            b_t = const.tile([OUT, 1], F32)
            nc.sync.dma_start(out=b_t[:, :], in_=bvec[:, :])

            # persistent big tensors
            y_all = const.tile([128, (N // 128) * OUT], F32R)  # Y chunks side by side
            embT = const.tile([OUT, LOC], F32)  # local emb.T
            sq_row = const.tile([1, LOC], F32)
            aT = const.tile([OUT + 2, LOC], F32)  # [-2 emb.T; 1; sq+EPS]
            bT = const.tile([OUT + 2, N], F32)  # [emb.T full; sq full; 1]

            # ---- stage 1: Y = X @ W.T  (Y chunk kc lives at y_all[:, kc*OUT:...]) ----
            w_sb = st1.tile([OUT, IN], F32)
            nc.sync.dma_start(out=w_sb[:, :], in_=w[:, :])
            ps_w = st1ps.tile([IN, OUT], F32, bufs=1)
            nc.tensor.transpose(ps_w[:, :], w_sb[:, :], ident[:OUT, :OUT])
            wt = const.tile([IN, OUT], F32)  # W.T
            nc.scalar.copy(wt[:, :], ps_w[:, :])

            for kc in range(N // 128):
                x_t = st1.tile([128, IN], F32, tag="x_t")
                nc.sync.dma_start(out=x_t[:, :], in_=x[kc * 128 : (kc + 1) * 128, :])
                ps_xt = st1ps.tile([IN, 128], F32, tag="ps_xt")
                nc.tensor.transpose(ps_xt[:, :], x_t[:, :], ident[:, :])
                xt_t = st1.tile([IN, 128], F32, tag="xt_t")
                nc.scalar.copy(xt_t[:, :], ps_xt[:, :])
                ps_y = st1ps.tile([128, OUT], F32, tag="ps_y")
                nc.tensor.matmul(
                    ps_y[:, :], xt_t[:, :], wt[:, :], start=True, stop=True
                )
                nc.scalar.copy(y_all[:, kc * OUT : (kc + 1) * OUT], ps_y[:, :])

        # ---- stage 2: emb.T = Y.T @ F_local.T + b ----
        with (
            tc.tile_pool(name="fnat", bufs=2) as fnat_pool,
            tc.tile_pool(name="ft", bufs=3) as ft_pool,
            tc.tile_pool(name="ftps", bufs=3, space="PSUM") as ftps,
            tc.tile_pool(name="embps", bufs=1, space="PSUM") as embps,
        ):
            ps_emb = [
                embps.tile([OUT, 512], F32, tag=f"ps_emb{h}", name=f"ps_emb{h}")
                for h in range(2)
            ]
            KC10 = 8  # 1024-wide column panels of F
            for kc10 in range(KC10):
                fnat = []
                for ib in range(8):
                    t = fnat_pool.tile([128, 1024], F32R, tag=f"fnat{ib}")
                    nc.sync.dma_start(
                        out=t[:, :],
                        in_=f[
                            ib * 128 : (ib + 1) * 128,
                            kc10 * 1024 : (kc10 + 1) * 1024,
                        ],
                    )
                    fnat.append(t)
                for kk in range(8):
                    kc = kc10 * 8 + kk  # global 128-col chunk of F
                    for half in range(2):
                        ps_ft = ftps.tile([128, 512], F32R, tag="ps_ft")
                        for q in range(4):
                            nc.tensor.transpose(
                                ps_ft[:, q * 128 : (q + 1) * 128],
                                fnat[half * 4 + q][:, kk * 128 : (kk + 1) * 128],
                                ident_r[:, :],
                            )
                        ft_t = ft_pool.tile([128, 512], F32R, tag="ft_t")
                        nc.scalar.copy(ft_t[:, :], ps_ft[:, :])
                        nc.tensor.matmul(
                            ps_emb[half][:, :],
                            y_all[:, kc * OUT : (kc + 1) * OUT],
                            ft_t[:, :],
                            start=(kc == 0),
                            stop=(kc == N // 128 - 1),
                        )
            # bias + write emb.T
            for half in range(2):
                nc.scalar.activation(
                    embT[:, half * 512 : (half + 1) * 512],
                    ps_emb[half][:, :],
                    mybir.ActivationFunctionType.Identity,
                    bias=b_t[:, 0:1],
                    scale=1.0,
                )

        with (
            tc.tile_pool(name="mid", bufs=2) as mid,
            tc.tile_pool(name="midps", bufs=2, space="PSUM") as midps,
        ):
            # ---- sq + feat output + allgather ----
            sq_el = mid.tile([OUT, LOC], F32)
            nc.scalar.square(sq_el[:, :], embT[:, :])
            for h in range(2):
                ps_sq = midps.tile([1, 512], F32, tag="ps_sq")
                nc.tensor.matmul(
                    ps_sq[:, :],
                    ones_col[:, :],
                    sq_el[:, h * 512 : (h + 1) * 512],
                    start=True,
                    stop=True,
                )
                nc.scalar.copy(sq_row[:, h * 512 : (h + 1) * 512], ps_sq[:, :])

            # feat output: transpose emb.T back to [LOC, OUT]
            for fb in range(LOC // 128):
                ps_f = midps.tile([128, OUT], F32, tag="ps_f")
                nc.tensor.transpose(
                    ps_f[:, :],
                    embT[:, fb * 128 : (fb + 1) * 128],
                    ident[:OUT, :OUT],
                )
                f_sb = mid.tile([128, OUT], F32, tag="f_sb")
                nc.scalar.copy(f_sb[:, :], ps_f[:, :])
                nc.sync.dma_start(
                    out=feat[fb * 128 : (fb + 1) * 128, :], in_=f_sb[:, :]
                )

            # allgather [emb.T; sq]
            nc.sync.dma_start(out=ag_in[:OUT, :], in_=embT[:, :])
            nc.sync.dma_start(out=ag_in[OUT : OUT + 1, :], in_=sq_row[:, :])
            nc.gpsimd.collective_compute(
                "AllGather",
                mybir.AluOpType.bypass,
                replica_groups=[list(range(NCORES))],
                ins=[ag_in[:, :].opt()],
                outs=[ag_out[:, :, :].opt()],
            )
            # B = [emb.T full; sq full; 1]
            nc.sync.dma_start(
                out=bT[:OUT, :].rearrange("p (r c) -> p r c", r=NCORES),
                in_=ag_out[:, :OUT, :].rearrange("r p c -> p r c"),
            )
            nc.sync.dma_start(
                out=bT[OUT : OUT + 1, :].rearrange("p (r c) -> p r c", r=NCORES),
                in_=ag_out[:, OUT : OUT + 1, :].rearrange("r p c -> p r c"),
            )
            # engine writes need base partition in {0,32,64,96}; rows at 65 are
            # filled via SBUF->SBUF DMA from base-0 staging tiles instead.
            ones_n = mid.tile([1, N], F32)
            nc.vector.memset(ones_n[:, :], 1.0)
            nc.sync.dma_start(out=bT[OUT + 1 : OUT + 2, :], in_=ones_n[:, :])
            # A = [-2 emb.T local; 1; sq+EPS]
            nc.scalar.mul(aT[:OUT, :], embT[:, :], -2.0)
            nc.vector.memset(aT[OUT : OUT + 1, :], 1.0)
            sq_eps = mid.tile([1, LOC], F32)
            nc.vector.tensor_scalar_add(sq_eps[:, :], sq_row[:, :], EPS)
            nc.sync.dma_start(out=aT[OUT + 1 : OUT + 2, :], in_=sq_eps[:, :])

        # ---- stage 3: force = sqrt(A.T @ B) * p2p ----
        with (
            tc.tile_pool(name="p2pp", bufs=3) as p2pp,
            tc.tile_pool(name="dtl", bufs=3) as dtl,
            tc.tile_pool(name="otl", bufs=3) as otl,
            tc.tile_pool(name="d2ps", bufs=4, space="PSUM") as d2ps,
        ):
            for ib in range(LOC // 128):
                for jp in range(8):  # 1024-col output panels
                    p2p_t = p2pp.tile([128, 1024], F32, tag="p2p_t")
                    nc.sync.dma_start(
                        out=p2p_t[:, :],
                        in_=p2p[
                            ib * 128 : (ib + 1) * 128,
                            jp * 1024 : (jp + 1) * 1024,
                        ],
                    )
                    o_t = otl.tile([128, 1024], F32, tag="o_t")
                    for h in range(2):
                        j = jp * 2 + h
                        ps_d2 = d2ps.tile([128, 512], F32, tag="ps_d2")
                        nc.tensor.matmul(
                            ps_d2[:, :],
                            aT[:, ib * 128 : (ib + 1) * 128],
                            bT[:, j * 512 : (j + 1) * 512],
                            start=True,
                            stop=True,
                        )
                        d_t = dtl.tile([128, 512], F32, tag="d_t")
                        nc.scalar.sqrt(d_t[:, :], ps_d2[:, :])
                        nc.vector.tensor_mul(
                            o_t[:, h * 512 : (h + 1) * 512],
                            d_t[:, :],
                            p2p_t[:, h * 512 : (h + 1) * 512],
                        )
                    nc.sync.dma_start(
                        out=force[
                            ib * 128 : (ib + 1) * 128,
                            jp * 1024 : (jp + 1) * 1024,
                        ],
                        in_=o_t[:, :],
                    )

    nc.compile()
    return nc


def get_nc():
    if "nc" not in _cache:
        _cache["nc"] = _build()
    return _cache["nc"]


def make_in_maps(F_, X, p2p, W, b):
    F_ = np.ascontiguousarray(F_, dtype=np.float32)
    X = np.ascontiguousarray(X, dtype=np.float32)
    p2p = np.ascontiguousarray(p2p, dtype=np.float32)
    W = np.ascontiguousarray(W, dtype=np.float32)
    b = np.ascontiguousarray(b, dtype=np.float32).reshape(OUT, 1)
    return [
        {
            "f": F_[c * LOC : (c + 1) * LOC],
            "x": X,
            "w": W,
            "bvec": b,
            "p2p": p2p[c * LOC : (c + 1) * LOC],
        }
        for c in range(NCORES)
    ]


def kernel(F_, X, p2p, W, b):
    nc = get_nc()
    in_maps = make_in_maps(F_, X, p2p, W, b)
    res = run_bass_kernel_spmd(nc, in_maps, core_ids=list(range(NCORES)))
    new_force = np.concatenate([res.results[c]["force"] for c in range(NCORES)], 0)
    new_features = np.concatenate([res.results[c]["feat"] for c in range(NCORES)], 0)
    return new_force, new_features
